# revision 13
# baseline (speedup 1.0000x reference)
"""Trainium2 Bass kernel for nn_Enet_81037442941606 (gnn_message_passing).

Computation (reference):
    g   = enc_out[batch_idx, tgt]                      # [N, D] gather
    h0  = batchnorm(g)  (training stats, biased var)   # [N, D]
    h1  = swish(h0 @ wt2_w.T + wt2_b)                  # [N, C]
    out = h1 @ A.T + h1   (A sparse, NNZ entries)      # [N, C]

Strategy (8 NeuronCores, tensor parallel over the class axis):
  * Each core owns a contiguous block of C/8 = 8192 classes: its wt2_w rows,
    its A rows (spmm output rows), and its output columns.
  * Device: token gather, PE-transpose, batchnorm stats, bf16 main matmul
    producing the h1^T shard; the h1^T exchange is FOUR chunked bf16
    AllGathers (16 c-tiles each) fired as phase B completes each quarter,
    landing in two shared DRAM tensors ag_lo / ag_hi (32768 rows each).
  * Phase D (spmm + residual) runs in two passes: the LO pass consumes only
    ag_lo (overlapping the later AllGathers on the wire) and accumulates
    into the f32 h1^T residual buffer; the HI pass consumes ag_hi and
    writes the output shard. Row gathers use batched dma_gather (1024 rows
    per SWDGE instruction, 15-bit indices) feeding selection-matrix matmuls
    that accumulate in PSUM.
  * Host concatenates the 8 output shards and transposes back to [N, C].
"""

import numpy as np
import ml_dtypes

import concourse.bacc as bacc
import concourse.bass as bass
import concourse.mybir as mybir
import concourse.tile as tile
from concourse.bass_utils import run_bass_kernel_spmd
from concourse.masks import make_identity

# Problem sizes (hardcoded per contest rules).
B, S, D, C, N = 32, 128, 1024, 65536, 512
NNZ = 262144
EPS = 1e-5
NCORES = 8
CLOC = C // NCORES          # classes per core = 8192
NB = CLOC // 128            # 64 row-blocks per core
ND = D // 128               # 8 contraction chunks
NT = N // 128               # 4 token tiles
P = 128

EX_DT = mybir.dt.bfloat16   # h1 exchange dtype
EX_NP = ml_dtypes.bfloat16
MM_DT = mybir.dt.bfloat16   # main-matmul operand dtype (W, h0^T)
MM_NP = ml_dtypes.bfloat16

NAG = 4                     # AllGather chunks
AGT = NB // NAG             # 16 c-tiles per AG chunk
AGR = AGT * P               # 2048 rows per rank per AG chunk
HROWS = 2 * NCORES * AGR    # 32768 rows per half (2 AG chunks)

G = 8                       # chunks per dma_gather instruction (1024 rows)

_PROGRAM_CACHE = {}
TRACE = False          # set by test.py to capture an NTFF profile
LAST_RESULTS = None    # BassKernelResults of the last kernel() call


def _build_program(chunks_lo: tuple, chunks_hi: tuple):
    """Build + compile the SPMD Bass program (identical on all 8 cores).

    chunks_lo[rb] / chunks_hi[rb] = number of 128-contribution chunks of row
    block rb sourcing from ag_lo / ag_hi (same profile on every core; the
    per-core slot data is padded to it).
    """
    key = (chunks_lo, chunks_hi)
    if key in _PROGRAM_CACHE:
        return _PROGRAM_CACHE[key]
    tot_lo, tot_hi = sum(chunks_lo), sum(chunks_hi)
    tot_ch = tot_lo + tot_hi

    nc = bacc.Bacc("TRN2", target_bir_lowering=False, debug=False,
                   num_devices=NCORES)
    f32 = mybir.dt.float32
    i32 = mybir.dt.int32
    i16 = mybir.dt.int16

    enc = nc.dram_tensor("enc", [B * S, D], f32, kind="ExternalInput")
    gidx = nc.dram_tensor("gidx", [P, NT], i32, kind="ExternalInput")
    wt = nc.dram_tensor("wt", [NB, P, D], MM_DT, kind="ExternalInput")
    biasv = nc.dram_tensor("biasv", [P, NB], f32, kind="ExternalInput")
    sel = nc.dram_tensor("sel", [P, tot_ch * P], EX_DT, kind="ExternalInput")
    # per-chunk gather rows (15-bit, within ag_lo / ag_hi)
    gidxs = nc.dram_tensor("gidxs", [P, tot_ch], i32, kind="ExternalInput")
    outT = nc.dram_tensor("outT", [CLOC, N], f32, kind="ExternalOutput")

    # Four chunked AllGathers over class sub-ranges; chunks k land in
    # ag_lo (k=0,1) / ag_hi (k=2,3) with chunk-major row layout
    # row(k, rr, l) = (k%2)*16384 + rr*2048 + (l - k*2048). All phase-D
    # gather indices are therefore 15-bit.
    ag_ins = [nc.dram_tensor(f"ag_in{k}", [AGR, N], EX_DT) for k in range(NAG)]
    ag_lo = nc.dram_tensor("ag_lo", [HROWS, N], EX_DT, addr_space="Shared")
    ag_hi = nc.dram_tensor("ag_hi", [HROWS, N], EX_DT, addr_space="Shared")
    ag_outs = [ag_lo[0:NCORES * AGR, :], ag_lo[NCORES * AGR:HROWS, :],
               ag_hi[0:NCORES * AGR, :], ag_hi[NCORES * AGR:HROWS, :]]
    ag_in_vs = [t.ap().rearrange("(i p) n -> i p n", p=P) for t in ag_ins]
    outT_v = outT.ap().rearrange("(i p) n -> i p n", p=P)

    # flat chunk -> row-block maps
    lbl_lo = [rb for rb in range(NB) for _ in range(chunks_lo[rb])]
    lbl_hi = [rb for rb in range(NB) for _ in range(chunks_hi[rb])]
    first_lo = {}
    last_lo = {}
    for ch, rb in enumerate(lbl_lo):
        first_lo.setdefault(rb, ch)
        last_lo[rb] = ch
    first_hi = {}
    last_hi = {}
    for ch, rb in enumerate(lbl_hi):
        first_hi.setdefault(rb, ch)
        last_hi[rb] = ch

    with tile.TileContext(nc) as tc:
        with (
            tc.tile_pool(name="persist", bufs=1) as persist,
        ):
            # ---------------- Phase A: gather + batchnorm stats + h0^T -----
            h0T = persist.tile([P, ND * N], MM_DT)      # [d%128, (dchunk, n)]
            h1T = persist.tile([P, NB * N], f32)        # [c%128, (ctile, n)]
            bias_t = persist.tile([P, NB], f32)
            gidxs_t = persist.tile([P, tot_ch], i32)
            ident = persist.tile([P, P], f32)
            mean_s = persist.tile([P, ND], f32)
            rstd_s = persist.tile([P, ND], f32)

            make_identity(nc, ident[:])
            nc.sync.dma_start(out=bias_t[:], in_=biasv[:])
            nc.sync.dma_start(out=gidxs_t[:], in_=gidxs[:])

            gidx_t = persist.tile([P, NT], i32)
            nc.sync.dma_start(out=gidx_t[:], in_=gidx[:])

            with (
                tc.tile_pool(name="phA", bufs=1) as phA,
                tc.tile_pool(name="psA", bufs=4, space="PSUM") as psA,
            ):
                g_tiles = []
                for j in range(NT):
                    g_j = phA.tile([P, D], f32, tag=f"g{j}")
                    nc.gpsimd.indirect_dma_start(
                        out=g_j[:], out_offset=None, in_=enc[:],
                        in_offset=bass.IndirectOffsetOnAxis(
                            ap=gidx_t[:, j:j + 1], axis=0),
                    )
                    g_tiles.append(g_j)

                # Raw transpose g -> h0T (tokens on the free axis)
                for j in range(NT):
                    for i in range(ND):
                        tp = psA.tile([P, P], f32, space="PSUM", tag="tp")
                        nc.tensor.transpose(
                            tp[:], g_tiles[j][:, i * P:(i + 1) * P], ident[:])
                        nc.vector.tensor_copy(
                            out=h0T[:, i * N + j * P: i * N + (j + 1) * P],
                            in_=tp[:])

                # Batch stats along the free (token) axis via ACT accum_out
                sum_s = phA.tile([P, ND], f32, tag="sums")
                sq_s = phA.tile([P, ND], f32, tag="sqs")
                scr = phA.tile([P, N], f32, tag="scr")
                for i in range(ND):
                    nc.scalar.activation(
                        scr[:], h0T[:, i * N:(i + 1) * N],
                        mybir.ActivationFunctionType.Copy,
                        accum_out=sum_s[:, i:i + 1])
                    nc.scalar.activation(
                        scr[:], h0T[:, i * N:(i + 1) * N],
                        mybir.ActivationFunctionType.Square,
                        accum_out=sq_s[:, i:i + 1])

                ex2_s = phA.tile([P, ND], f32, tag="ex2")
                var_s = phA.tile([P, ND], f32, tag="var")
                nc.scalar.mul(mean_s[:], sum_s[:], 1.0 / N)
                nc.scalar.mul(ex2_s[:], sq_s[:], 1.0 / N)
                # var = E[x^2] - mean^2 ; rstd = 1/sqrt(var + eps)
                nc.vector.tensor_tensor(
                    out=var_s[:], in0=mean_s[:], in1=mean_s[:],
                    op=mybir.AluOpType.mult)
                nc.vector.tensor_tensor(
                    out=var_s[:], in0=ex2_s[:], in1=var_s[:],
                    op=mybir.AluOpType.subtract)
                sd_s = phA.tile([P, ND], f32, tag="sd")
                epsb = phA.tile([P, 1], f32, tag="epsb")
                nc.vector.memset(epsb[:], EPS)
                nc.scalar.activation(
                    sd_s[:], var_s[:], mybir.ActivationFunctionType.Sqrt,
                    bias=epsb[:, :1], scale=1.0)
                nc.vector.reciprocal(rstd_s[:], sd_s[:])

                # normalize h0T in place, one wide op per d-chunk
                for i in range(ND):
                    nc.vector.tensor_scalar(
                        out=h0T[:, i * N:(i + 1) * N],
                        in0=h0T[:, i * N:(i + 1) * N],
                        scalar1=mean_s[:, i:i + 1],
                        scalar2=rstd_s[:, i:i + 1],
                        op0=mybir.AluOpType.subtract,
                        op1=mybir.AluOpType.mult,
                    )

            # ---------------- Phase B: h1^T = swish(W h0^T + b) ------------
            # W loads batched 4 c-tiles per DMA (1 MB) to keep PE fed.
            WB = 4
            wt_b = wt.ap().rearrange("(a b) p d -> a b p d", b=WB)
            with (
                tc.tile_pool(name="phB", bufs=3) as phB,
                tc.tile_pool(name="psB", bufs=4, space="PSUM") as psB,
            ):
                for a in range(NB // WB):
                    wt_a = phB.tile([P, WB * D], MM_DT, tag="wt")
                    nc.sync.dma_start(
                        out=wt_a[:].rearrange("p (b d) -> p b d", b=WB),
                        in_=wt_b[a].rearrange("b p d -> p b d"))
                    for bsub in range(WB):
                        i = a * WB + bsub
                        h1ps = psB.tile([P, N], f32, space="PSUM", tag="h1ps")
                        for k in range(ND):
                            nc.tensor.matmul(
                                out=h1ps[:],
                                lhsT=wt_a[:, bsub * D + k * P:
                                          bsub * D + (k + 1) * P],
                                rhs=h0T[:, k * N:(k + 1) * N],
                                start=(k == 0), stop=(k == ND - 1),
                            )
                        nc.scalar.activation(
                            h1T[:, i * N:(i + 1) * N], h1ps[:],
                            mybir.ActivationFunctionType.Silu,
                            bias=bias_t[:, i:i + 1], scale=1.0)
                        h1bf = phB.tile([P, N], EX_DT, tag="h1bf")
                        nc.vector.tensor_copy(
                            out=h1bf[:], in_=h1T[:, i * N:(i + 1) * N])
                        nc.sync.dma_start(
                            out=ag_in_vs[i // AGT][i % AGT],
                            in_=h1bf[:])

                    # ---- Phase C: chunked AllGathers fire as soon as their
                    # class sub-range of h1^T has been written out.
                    if (a + 1) % (AGT // WB) == 0:
                        k = (a + 1) // (AGT // WB) - 1
                        nc.gpsimd.collective_compute(
                            "AllGather",
                            mybir.AluOpType.bypass,
                            replica_groups=[list(range(NCORES))],
                            ins=[ag_ins[k][:].opt()],
                            outs=[ag_outs[k].opt()],
                        )

            # ---------------- Phase D: spmm + residual, two passes ---------
            def spmm_pass(src_ag, lbl, first, last, ch_base, sel_off, hi):
                tot = len(lbl)
                accs = {}
                sel_t = None
                for ch in range(tot):
                    ct = ctp.tile([P, N], EX_DT, tag="ct")
                    nc.gpsimd.indirect_dma_start(
                        out=ct[:], out_offset=None, in_=src_ag[:],
                        in_offset=bass.IndirectOffsetOnAxis(
                            ap=gidxs_t[:, ch_base + ch:ch_base + ch + 1],
                            axis=0),
                    )
                    j = ch % G
                    if j == 0:
                        gsz = min(G, tot - ch)
                        sel_t = selp.tile([P, G * P], EX_DT, tag="sel")
                        nc.sync.dma_start(
                            out=sel_t[:, :gsz * P],
                            in_=sel[:, (sel_off + ch) * P:
                                    (sel_off + ch + gsz) * P])
                    rb = lbl[ch]
                    if ch == first[rb]:
                        acc_t = psD.tile([P, N], f32, space="PSUM", tag="acc")
                        accs[rb] = acc_t
                    nc.tensor.matmul(
                        out=accs[rb][:],
                        lhsT=sel_t[:, j * P:(j + 1) * P],
                        rhs=ct[:],
                        start=(ch == first[rb]), stop=(ch == last[rb]),
                    )
                    if ch == last[rb]:
                        if not hi:
                            # h1T[rb] += spmm_lo[rb]
                            nc.vector.tensor_tensor(
                                out=h1T[:, rb * N:(rb + 1) * N],
                                in0=accs[rb][:],
                                in1=h1T[:, rb * N:(rb + 1) * N],
                                op=mybir.AluOpType.add)
                        else:
                            o_t = otp.tile([P, N], f32, tag="ot")
                            nc.vector.tensor_tensor(
                                out=o_t[:], in0=accs[rb][:],
                                in1=h1T[:, rb * N:(rb + 1) * N],
                                op=mybir.AluOpType.add)
                            nc.sync.dma_start(out=outT_v[rb], in_=o_t[:])
                        del accs[rb]

            with (
                tc.tile_pool(name="ctp", bufs=14) as ctp,
                tc.tile_pool(name="selp", bufs=3) as selp,
                tc.tile_pool(name="otp", bufs=3) as otp,
                tc.tile_pool(name="psD", bufs=4, space="PSUM") as psD,
            ):
                spmm_pass(ag_lo, lbl_lo, first_lo, last_lo, 0, 0, hi=False)
                spmm_pass(ag_hi, lbl_hi, first_hi, last_hi, tot_lo, tot_lo,
                          hi=True)
                # row blocks with no hi-sourced contributions still need
                # their (residual + lo-pass) output written out
                for rb in range(NB):
                    if chunks_hi[rb] == 0:
                        nc.sync.dma_start(out=outT_v[rb],
                                          in_=h1T[:, rb * N:(rb + 1) * N])

    nc.compile()
    _PROGRAM_CACHE[key] = nc
    return nc


def _prep_host(enc_out, wt2_w, wt2_b, A_values, batch_idx, tgt, A_indices):
    """Shard inputs + restructure the sparse matrix for the device program."""
    enc_flat = np.ascontiguousarray(
        np.asarray(enc_out, dtype=np.float32).reshape(B * S, D))
    flat_idx = (np.asarray(batch_idx, dtype=np.int64) * S
                + np.asarray(tgt, dtype=np.int64)).astype(np.int32)
    gidx_host = np.ascontiguousarray(flat_idx.reshape(NT, P).T)

    wt2_w = np.asarray(wt2_w, dtype=np.float32)
    wt2_b = np.asarray(wt2_b, dtype=np.float32)
    rows_all = np.asarray(A_indices[0], dtype=np.int64)
    cols_all = np.asarray(A_indices[1], dtype=np.int64)
    vals_all = np.asarray(A_values, dtype=np.float32)

    # Global gather-row id for source class c (owner rank rr, local l):
    # AG chunk k = l // AGR, half = k // 2,
    # row15 = (k % 2)*16384 + rr*2048 + (l % AGR)   (15-bit)
    rr_all = cols_all // CLOC
    l_all = cols_all % CLOC
    k_all = l_all // AGR
    half_all = k_all // 2
    row15_all = (k_all % 2) * (NCORES * AGR) + rr_all * AGR + (l_all % AGR)

    # Per-rank slot construction. Slots sorted by (rb, half, then stable);
    # chunk profile = max over ranks of per-(rb, half) chunk counts.
    per_rank_raw = []
    nlo = np.zeros((NCORES, NB), dtype=np.int64)
    nhi = np.zeros((NCORES, NB), dtype=np.int64)
    for r in range(NCORES):
        m = (rows_all // CLOC) == r
        rl = rows_all[m] - r * CLOC          # local out row
        vv = vals_all[m]
        r15 = row15_all[m]
        hf = half_all[m]
        rb = rl // P
        order = np.lexsort((hf, rb))
        rl, vv, r15, hf, rb = (rl[order], vv[order], r15[order],
                               hf[order], rb[order])
        per_rank_raw.append((rl, vv, r15, hf, rb))
        for b in range(NB):
            mb = rb == b
            nlo[r, b] = int((hf[mb] == 0).sum())
            nhi[r, b] = int((hf[mb] == 1).sum())

    chunks_lo = tuple(int(x) for x in
                      np.ceil(nlo.max(axis=0) / P).astype(np.int64))
    chunks_hi = tuple(int(x) for x in
                      np.ceil(nhi.max(axis=0) / P).astype(np.int64))
    tot_lo, tot_hi = sum(chunks_lo), sum(chunks_hi)
    tot_ch = tot_lo + tot_hi
    off_lo = np.zeros(NB, dtype=np.int64)
    off_lo[1:] = np.cumsum(chunks_lo)[:-1]
    off_hi = np.zeros(NB, dtype=np.int64)
    off_hi[1:] = np.cumsum(chunks_hi)[:-1]

    per_rank = []
    for r in range(NCORES):
        rl, vv, r15, hf, rb = per_rank_raw[r]
        # global flat chunk id and slot within chunk for each contribution
        pos_in = np.zeros(len(rl), dtype=np.int64)
        chunk_of = np.zeros(len(rl), dtype=np.int64)
        for b in range(NB):
            mlo = (rb == b) & (hf == 0)
            mhi = (rb == b) & (hf == 1)
            pos_in[mlo] = np.arange(mlo.sum())
            pos_in[mhi] = np.arange(mhi.sum())
            chunk_of[mlo] = off_lo[b] + pos_in[mlo] // P
            chunk_of[mhi] = tot_lo + off_hi[b] + pos_in[mhi] // P
        p_idx = pos_in % P

        sel_host = np.zeros((P, tot_ch * P), dtype=EX_NP)
        sel_host[p_idx, chunk_of * P + (rl % P)] = vv.astype(EX_NP)

        # per-chunk gather rows: slot (ch, p) reads ag half row gidxs[p, ch]
        gidxs_host = np.zeros((P, tot_ch), dtype=np.int32)
        gidxs_host[p_idx, chunk_of] = r15.astype(np.int32)

        rows = slice(r * CLOC, (r + 1) * CLOC)
        wr = wt2_w[rows]                            # [8192, 1024]
        wt_host = np.ascontiguousarray(
            wr.reshape(NB, P, ND, P).transpose(0, 3, 2, 1)
        ).reshape(NB, P, D).astype(MM_NP)
        bias_host = np.ascontiguousarray(wt2_b[rows].reshape(NB, P).T)
        per_rank.append({
            "enc": enc_flat,
            "gidx": gidx_host,
            "wt": wt_host,
            "biasv": bias_host,
            "sel": sel_host,
            "gidxs": gidxs_host,
        })
    return per_rank, chunks_lo, chunks_hi


def kernel(**inputs) -> np.ndarray:
    per_rank, chunks_lo, chunks_hi = _prep_host(
        inputs["enc_out"], inputs["wt2_w"], inputs["wt2_b"],
        inputs["A_values"], inputs["batch_idx"], inputs["tgt"],
        inputs["A_indices"])
    nc = _build_program(chunks_lo, chunks_hi)
    res = None
    last_exc = None
    for _attempt in range(3):
        try:
            res = run_bass_kernel_spmd(
                nc, per_rank, core_ids=list(range(NCORES)), trace=TRACE)
            break
        except Exception as e:  # transient runtime/collective hiccups
            last_exc = e
    if res is None:
        raise last_exc
    global LAST_RESULTS
    LAST_RESULTS = res
    outT_full = np.empty((C, N), dtype=np.float32)
    for r in range(NCORES):
        outT_full[r * CLOC:(r + 1) * CLOC] = res.results[r]["outT"]
    return np.ascontiguousarray(outT_full.T)


# revision 19
# speedup vs baseline: 1.2690x; 1.2690x over previous
"""Trainium2 Bass kernel for nn_Enet_81037442941606 (gnn_message_passing).

Computation (reference):
    g   = enc_out[batch_idx, tgt]                      # [N, D] gather
    h0  = batchnorm(g)  (training stats, biased var)   # [N, D]
    h1  = swish(h0 @ wt2_w.T + wt2_b)                  # [N, C]
    out = h1 @ A.T + h1   (A sparse, NNZ entries)      # [N, C]

Strategy (8 NeuronCores, tensor parallel over the class axis):
  * Each core owns a contiguous block of C/8 = 8192 classes: its wt2_w rows,
    its A rows (spmm output rows), and its output columns.
  * Host pre-transposes the W shard so the device reads perfect [d, c] tiles,
    and packs the sparse matrix as per-row-block selection matrices + column
    gather indices (pure data-layout transforms of A).
  * Device: token gather, PE-transpose of the activations, batchnorm stats
    along the free axis (ACT accum_out), in-place normalize; bf16 main matmul
    producing the h1^T shard (f32 resident in SBUF); two chunked bf16
    AllGathers overlapping the matmul tail; then the spmm as indirect
    row-gathers from the gathered h1^T feeding selection-matrix matmuls that
    accumulate in PSUM, fused f32 residual add, transposed output shard out.
  * Host concatenates the 8 output shards and transposes back to [N, C].
"""

import math

import numpy as np
import ml_dtypes

import concourse.bacc as bacc
import concourse.bass as bass
import concourse.mybir as mybir
import concourse.tile as tile
from concourse.bass_utils import run_bass_kernel_spmd
from concourse.masks import make_identity

# Problem sizes (hardcoded per contest rules).
B, S, D, C, N = 32, 128, 1024, 65536, 512
NNZ = 262144
EPS = 1e-5
NCORES = 8
CLOC = C // NCORES          # classes per core = 8192
NB = CLOC // 128            # 64 row-blocks per core
ND = D // 128               # 8 contraction chunks
NT = N // 128               # 4 token tiles
P = 128

EX_DT = mybir.dt.bfloat16   # h1 exchange dtype
EX_NP = ml_dtypes.bfloat16
MM_DT = mybir.dt.bfloat16   # main-matmul operand dtype (W, h0^T)
MM_NP = ml_dtypes.bfloat16
AG_SPLIT = 24               # c-tiles in the first (smaller) AllGather

_PROGRAM_CACHE = {}
TRACE = False          # set by test.py to capture an NTFF profile
LAST_RESULTS = None    # BassKernelResults of the last kernel() call


def _build_program(chunks: tuple):
    """Build + compile the SPMD Bass program (identical on all 8 cores).

    chunks[rb] = number of 128-contribution gather/matmul chunks for row
    block rb (same profile on every core; per-core data is padded to it).
    """
    if chunks in _PROGRAM_CACHE:
        return _PROGRAM_CACHE[chunks]
    tot_ch = sum(chunks)

    nc = bacc.Bacc("TRN2", target_bir_lowering=False, debug=False,
                   num_devices=NCORES)
    f32 = mybir.dt.float32
    i32 = mybir.dt.int32

    enc = nc.dram_tensor("enc", [B * S, D], f32, kind="ExternalInput")
    gidx = nc.dram_tensor("gidx", [P, NT], i32, kind="ExternalInput")
    wt = nc.dram_tensor("wt", [NB, P, D], MM_DT, kind="ExternalInput")
    biasv = nc.dram_tensor("biasv", [P, NB], f32, kind="ExternalInput")
    sel = nc.dram_tensor("sel", [P, tot_ch * P], EX_DT, kind="ExternalInput")
    gidxs = nc.dram_tensor("gidxs", [P, tot_ch], i32, kind="ExternalInput")
    outT = nc.dram_tensor("outT", [CLOC, N], EX_DT, kind="ExternalOutput")

    # AllGather is split into NAG chunked collectives over class sub-ranges so
    # the first chunks overlap the tail of the main matmul. Each chunk's
    # output is a strided slice of the single ag_out tensor, keeping one
    # uniform global row index space for the spmm gathers.
    CCHS = [AG_SPLIT * P, CLOC - AG_SPLIT * P]   # uneven split (tiles 0:24, 24:64)
    ag_ins = [nc.dram_tensor(f"ag_in{k}", [CCHS[k], N], EX_DT) for k in range(2)]
    ag_out = nc.dram_tensor("ag_out", [C, N], EX_DT, addr_space="Shared")
    # ag_out row space is chunk-major: chunk 0 rows [0, 8*CCHS[0]) laid
    # rr*CCHS[0]+l, chunk 1 rows 8*CCHS[0] + rr*CCHS[1] + (l-CCHS[0]).
    # Host remaps gather indices to this layout.
    b0 = NCORES * CCHS[0]
    ag_out_ch = [ag_out[0:b0, :], ag_out[b0:C, :]]
    ag_in_vs = [t.ap().rearrange("(i p) n -> i p n", p=P) for t in ag_ins]
    outT_v = outT.ap().rearrange("(i p) n -> i p n", p=P)

    with tile.TileContext(nc) as tc:
        with (
            tc.tile_pool(name="persist", bufs=1) as persist,
        ):
            # ---------------- Phase A: gather + batchnorm stats + h0^T -----
            h0T = persist.tile([P, ND * N], MM_DT)      # [d%128, (dchunk, n)]
            h1T = persist.tile([P, NB * N], f32)        # [c%128, (ctile, n)]
            bias_t = persist.tile([P, NB], f32)
            gidxs_t = persist.tile([P, tot_ch], i32)
            ident = persist.tile([P, P], f32)
            ones = persist.tile([P, 1], f32)
            mean_s = persist.tile([P, ND], f32)
            rstd_s = persist.tile([P, ND], f32)

            make_identity(nc, ident[:])
            nc.vector.memset(ones[:], 1.0)
            nc.sync.dma_start(out=bias_t[:], in_=biasv[:])
            nc.sync.dma_start(out=gidxs_t[:], in_=gidxs[:])

            gidx_t = persist.tile([P, NT], i32)
            nc.sync.dma_start(out=gidx_t[:], in_=gidx[:])

            with (
                tc.tile_pool(name="phA", bufs=1) as phA,
                tc.tile_pool(name="psA", bufs=4, space="PSUM") as psA,
            ):
                g_tiles = []
                for j in range(NT):
                    g_j = phA.tile([P, D], f32, tag=f"g{j}")
                    nc.gpsimd.indirect_dma_start(
                        out=g_j[:], out_offset=None, in_=enc[:],
                        in_offset=bass.IndirectOffsetOnAxis(
                            ap=gidx_t[:, j:j + 1], axis=0),
                    )
                    g_tiles.append(g_j)

                # Raw transpose g -> h0T (tokens on the free axis)
                for j in range(NT):
                    for i in range(ND):
                        tp = psA.tile([P, P], f32, space="PSUM", tag="tp")
                        nc.tensor.transpose(
                            tp[:], g_tiles[j][:, i * P:(i + 1) * P], ident[:])
                        nc.vector.tensor_copy(
                            out=h0T[:, i * N + j * P: i * N + (j + 1) * P],
                            in_=tp[:])

                # Batch stats along the free (token) axis via ACT accum_out
                sum_s = phA.tile([P, ND], f32, tag="sums")
                sq_s = phA.tile([P, ND], f32, tag="sqs")
                scr = phA.tile([P, N], f32, tag="scr")
                for i in range(ND):
                    nc.scalar.activation(
                        scr[:], h0T[:, i * N:(i + 1) * N],
                        mybir.ActivationFunctionType.Copy,
                        accum_out=sum_s[:, i:i + 1])
                    nc.scalar.activation(
                        scr[:], h0T[:, i * N:(i + 1) * N],
                        mybir.ActivationFunctionType.Square,
                        accum_out=sq_s[:, i:i + 1])

                ex2_s = phA.tile([P, ND], f32, tag="ex2")
                var_s = phA.tile([P, ND], f32, tag="var")
                nc.scalar.mul(mean_s[:], sum_s[:], 1.0 / N)
                nc.scalar.mul(ex2_s[:], sq_s[:], 1.0 / N)
                # var = E[x^2] - mean^2 ; rstd = 1/sqrt(var + eps)
                nc.vector.tensor_tensor(
                    out=var_s[:], in0=mean_s[:], in1=mean_s[:],
                    op=mybir.AluOpType.mult)
                nc.vector.tensor_tensor(
                    out=var_s[:], in0=ex2_s[:], in1=var_s[:],
                    op=mybir.AluOpType.subtract)
                sd_s = phA.tile([P, ND], f32, tag="sd")
                epsb = phA.tile([P, 1], f32, tag="epsb")
                nc.vector.memset(epsb[:], EPS)
                nc.scalar.activation(
                    sd_s[:], var_s[:], mybir.ActivationFunctionType.Sqrt,
                    bias=epsb[:, :1], scale=1.0)
                nc.vector.reciprocal(rstd_s[:], sd_s[:])

                # normalize h0T in place, one wide op per d-chunk
                for i in range(ND):
                    nc.vector.tensor_scalar(
                        out=h0T[:, i * N:(i + 1) * N],
                        in0=h0T[:, i * N:(i + 1) * N],
                        scalar1=mean_s[:, i:i + 1],
                        scalar2=rstd_s[:, i:i + 1],
                        op0=mybir.AluOpType.subtract,
                        op1=mybir.AluOpType.mult,
                    )

            # ---------------- Phase B: h1^T = swish(W h0^T + b) ------------
            # W loads batched 4 c-tiles per DMA (1 MB) to keep PE fed.
            WB = 4
            wt_b = wt.ap().rearrange("(a b) p d -> a b p d", b=WB)
            with (
                tc.tile_pool(name="phB", bufs=3) as phB,
                tc.tile_pool(name="psB", bufs=4, space="PSUM") as psB,
            ):
                for a in range(NB // WB):
                    wt_a = phB.tile([P, WB * D], MM_DT, tag="wt")
                    nc.sync.dma_start(
                        out=wt_a[:].rearrange("p (b d) -> p b d", b=WB),
                        in_=wt_b[a].rearrange("b p d -> p b d"))
                    for bsub in range(WB):
                        i = a * WB + bsub
                        h1ps = psB.tile([P, N], f32, space="PSUM", tag="h1ps")
                        for k in range(ND):
                            nc.tensor.matmul(
                                out=h1ps[:],
                                lhsT=wt_a[:, bsub * D + k * P:
                                          bsub * D + (k + 1) * P],
                                rhs=h0T[:, k * N:(k + 1) * N],
                                start=(k == 0), stop=(k == ND - 1),
                            )
                        nc.scalar.activation(
                            h1T[:, i * N:(i + 1) * N], h1ps[:],
                            mybir.ActivationFunctionType.Silu,
                            bias=bias_t[:, i:i + 1], scale=1.0)
                        h1bf = phB.tile([P, N], EX_DT, tag="h1bf")
                        nc.vector.tensor_copy(
                            out=h1bf[:], in_=h1T[:, i * N:(i + 1) * N])
                        q = 0 if i < AG_SPLIT else 1
                        nc.sync.dma_start(
                            out=ag_in_vs[q][i - (AG_SPLIT if q else 0)],
                            in_=h1bf[:])

                    # ---- Phase C: chunked AllGathers fire as soon as their
                    # class sub-range of h1^T has been written out.
                    if (a + 1) * WB == AG_SPLIT:
                        k = 0
                    elif (a + 1) * WB == NB:
                        k = 1
                    else:
                        k = None
                    if k is not None:
                        nc.gpsimd.collective_compute(
                            "AllGather",
                            mybir.AluOpType.bypass,
                            replica_groups=[list(range(NCORES))],
                            ins=[ag_ins[k][:].opt()],
                            outs=[ag_out_ch[k].opt()],
                        )

            # ---------------- Phase D: spmm + residual ---------------------
            with (
                tc.tile_pool(name="phD", bufs=3) as phD,
                tc.tile_pool(name="ctp", bufs=20) as ctp,
                tc.tile_pool(name="psD", bufs=3, space="PSUM") as psD,
            ):
                off = 0
                for rb in range(NB):
                    nch_b = chunks[rb]
                    sel_t = phD.tile([P, max(chunks) * P], EX_DT, tag="sel")
                    nc.sync.dma_start(
                        out=sel_t[:, :nch_b * P],
                        in_=sel[:, off * P:(off + nch_b) * P])
                    acc = psD.tile([P, N], f32, space="PSUM", tag="acc")
                    for ch in range(nch_b):
                        ct = ctp.tile([P, N], EX_DT, tag="ct")
                        nc.gpsimd.indirect_dma_start(
                            out=ct[:], out_offset=None, in_=ag_out[:],
                            in_offset=bass.IndirectOffsetOnAxis(
                                ap=gidxs_t[:, off + ch:off + ch + 1],
                                axis=0),
                        )
                        nc.tensor.matmul(
                            out=acc[:],
                            lhsT=sel_t[:, ch * P:(ch + 1) * P],
                            rhs=ct[:],
                            start=(ch == 0), stop=(ch == nch_b - 1),
                        )
                    o_t = phD.tile([P, N], EX_DT, tag="ot")
                    nc.vector.tensor_tensor(
                        out=o_t[:], in0=acc[:],
                        in1=h1T[:, rb * N:(rb + 1) * N],
                        op=mybir.AluOpType.add)
                    nc.sync.dma_start(out=outT_v[rb], in_=o_t[:])
                    off += nch_b

    nc.compile()
    _PROGRAM_CACHE[chunks] = nc
    return nc


def _prep_host(enc_out, wt2_w, wt2_b, A_values, batch_idx, tgt, A_indices):
    """Shard inputs + restructure the sparse matrix for the device program."""
    enc_flat = np.ascontiguousarray(
        np.asarray(enc_out, dtype=np.float32).reshape(B * S, D))
    flat_idx = (np.asarray(batch_idx, dtype=np.int64) * S
                + np.asarray(tgt, dtype=np.int64)).astype(np.int32)
    gidx_host = np.ascontiguousarray(flat_idx.reshape(NT, P).T)

    wt2_w = np.asarray(wt2_w, dtype=np.float32)
    wt2_b = np.asarray(wt2_b, dtype=np.float32)
    rows_all = np.asarray(A_indices[0], dtype=np.int64)
    cols_all = np.asarray(A_indices[1], dtype=np.int64)
    vals_all = np.asarray(A_values, dtype=np.float32)

    # Per-rank sparse slices + row degrees.
    rank_data = []
    for r in range(NCORES):
        m = (rows_all // CLOC) == r
        rl = (rows_all[m] - r * CLOC).astype(np.int64)
        cc = cols_all[m]
        vv = vals_all[m]
        deg = np.bincount(rl, minlength=CLOC)
        rank_data.append((rl, cc, vv, deg))

    # Pick a global per-block chunk profile: most blocks get 4 chunks
    # (512 contribution slots), NFAT fat blocks (at the end) get 5. Rows are
    # packed into blocks degree-aware so every block fits its capacity.
    # This is a pure relabeling of the class axis within each core: W rows,
    # bias, h1^T tiles, ag_out rows and the final output rows all follow the
    # same permutation (undone on the host at the end).
    max_nnz = max(len(rd[0]) for rd in rank_data)
    base = max(1, max_nnz // (NB * P))   # baseline chunks per block
    nfat = 6                             # fat blocks absorb the remainder
    while True:
        caps = np.full(NB, base * P, dtype=np.int64)
        caps[NB - nfat:] = (base + 1) * P
        perms = []
        ok = True
        for r in range(NCORES):
            deg = rank_data[r][3]
            order = np.argsort(-deg, kind="stable")
            loads = np.zeros(NB, dtype=np.int64)
            cnts = np.zeros(NB, dtype=np.int64)
            assign = np.empty(CLOC, dtype=np.int64)   # row -> bin
            slot = np.empty(CLOC, dtype=np.int64)     # row -> slot in bin
            for row in order:
                d = deg[row]
                score = (loads + d) / caps
                score[cnts >= P] = np.inf
                score[loads + d > caps] = np.inf
                b = int(np.argmin(score))
                if not np.isfinite(score[b]):
                    ok = False
                    break
                assign[row] = b
                slot[row] = cnts[b]
                loads[b] += d
                cnts[b] += 1
            if not ok:
                break
            old2new = assign * P + slot               # old local -> new local
            perms.append(old2new)
        if ok:
            break
        nfat += 4
        if nfat > NB:
            raise RuntimeError("packing failed")
    chunks = tuple(int(caps[rb] // P) for rb in range(NB))
    tot_ch = sum(chunks)
    ch_off = np.zeros(NB, dtype=np.int64)
    ch_off[1:] = np.cumsum(chunks)[:-1]

    NAG = 2
    new2old = [np.argsort(p) for p in perms]

    per_rank = []
    for r in range(NCORES):
        rl, cc, vv, _deg = rank_data[r]
        rl_new = perms[r][rl]
        order = np.argsort(rl_new, kind="stable")
        rl_new, cc, vv = rl_new[order], cc[order], vv[order]
        blk = rl_new // P
        counts = np.bincount(blk, minlength=NB)
        starts = np.zeros(NB, dtype=np.int64)
        starts[1:] = np.cumsum(counts)[:-1]
        pos = np.arange(len(rl_new)) - starts[blk]
        ch_idx = pos // P
        # gather row for class c (owner rank rr, old local l): apply rr's
        # permutation, then the uneven chunk-major ag_out layout of the two
        # chunked AllGathers.
        rr = cc // CLOC
        lnew = np.empty(len(cc), dtype=np.int64)
        for r2 in range(NCORES):
            m2 = rr == r2
            lnew[m2] = perms[r2][cc[m2] % CLOC]
        b0l = AG_SPLIT * P
        ag_row = np.where(
            lnew < b0l,
            rr * b0l + lnew,
            NCORES * b0l + rr * (CLOC - b0l) + (lnew - b0l))
        # re-sort slots within each chunk by ascending gather row so the
        # chunk's 128 DMA descriptors hit HBM in address order
        chunk_g = ch_off[blk] + ch_idx
        ord2 = np.lexsort((ag_row, chunk_g))
        rl_new, vv, ag_row, chunk_g = (rl_new[ord2], vv[ord2],
                                       ag_row[ord2], chunk_g[ord2])
        ccounts = np.bincount(chunk_g, minlength=tot_ch)
        cstarts = np.zeros(tot_ch, dtype=np.int64)
        cstarts[1:] = np.cumsum(ccounts)[:-1]
        p_idx = np.arange(len(rl_new)) - cstarts[chunk_g]
        sel_host = np.zeros((P, tot_ch * P), dtype=EX_NP)
        gidxs_host = np.zeros((P, tot_ch), dtype=np.int32)
        sel_host[p_idx, chunk_g * P + (rl_new % P)] = vv.astype(EX_NP)
        gidxs_host[p_idx, chunk_g] = ag_row.astype(np.int32)

        rows = slice(r * CLOC, (r + 1) * CLOC)
        wr = wt2_w[rows][new2old[r]]  # [8192, 1024] in permuted order
        wt_host = np.ascontiguousarray(
            wr.reshape(NB, P, ND, P).transpose(0, 3, 2, 1)
        ).reshape(NB, P, D).astype(MM_NP)
        bias_host = np.ascontiguousarray(
            wt2_b[rows][new2old[r]].reshape(NB, P).T)
        per_rank.append({
            "enc": enc_flat,
            "gidx": gidx_host,
            "wt": wt_host,
            "biasv": bias_host,
            "sel": sel_host,
            "gidxs": gidxs_host,
        })
    return per_rank, chunks, new2old


def kernel(**inputs) -> np.ndarray:
    per_rank, chunks, new2old = _prep_host(
        inputs["enc_out"], inputs["wt2_w"], inputs["wt2_b"],
        inputs["A_values"], inputs["batch_idx"], inputs["tgt"],
        inputs["A_indices"])
    nc = _build_program(chunks)
    res = None
    last_exc = None
    for _attempt in range(3):
        try:
            res = run_bass_kernel_spmd(
                nc, per_rank, core_ids=list(range(NCORES)), trace=TRACE)
            break
        except Exception as e:  # transient runtime/collective hiccups
            last_exc = e
    if res is None:
        raise last_exc
    global LAST_RESULTS
    LAST_RESULTS = res
    outT_full = np.empty((C, N), dtype=np.float32)
    for r in range(NCORES):
        outT_full[r * CLOC + new2old[r]] = \
            res.results[r]["outT"].astype(np.float32)
    return np.ascontiguousarray(outT_full.T)

